# revision 3
# baseline (speedup 1.0000x reference)
"""GQA attention (B=2,S=2048,D=2048,NH=16,NKV=4,DH=128, RoPE, causal) on 8 trn2 cores.

Sharding: core c = b*4+g handles batch b, kv-head g (4 q-heads).
Per-core: QKV projections (fp32r matmuls), RoPE, flash-style causal attention in
S^T layout (scores^T = K^T.T @ Q^T so softmax denominators come from a ones-matmul
and context^T needs no transposes), out-proj partial, then ReduceScatter(add) over
each batch's 4 cores; host assembles the full output.
"""
import sys, types
sys.path.insert(0, "/opt/trn_rl_repo")

import numpy as np
from contextlib import ExitStack


def _install_ntff_hook():
    try:
        import antenv.axon_hooks  # noqa
        return
    except ImportError:
        pass
    mod = types.ModuleType("antenv.axon_hooks")
    _h = [None]
    mod.set_axon_ntff_profile_hook = lambda h: _h.__setitem__(0, h)
    mod.get_axon_ntff_profile_hook = lambda: _h[0]
    sys.modules["antenv.axon_hooks"] = mod
    try:
        from trn_agent_boot.trn_boot import _ntff_profile_via_ctypes
        mod.set_axon_ntff_profile_hook(
            _ntff_profile_via_ctypes("/opt/axon/libaxon_pjrt.so"))
    except Exception:
        pass


_install_ntff_hook()

import concourse.bass as bass  # noqa: E402
import concourse.mybir as mybir  # noqa: E402
import concourse.tile as tile  # noqa: E402
from concourse import bacc, bass_utils  # noqa: E402
from concourse.masks import make_identity  # noqa: E402

bass_utils.upload_artifacts = lambda tmpdir: tmpdir  # no artifact share here

F32 = mybir.dt.float32
F32R = mybir.dt.float32r
AF = mybir.ActivationFunctionType
ALU = mybir.AluOpType

B, S, D = 2, 2048, 2048
NH, NKV, DH = 16, 4, 128
NG = NH // NKV          # 4 q heads per core
TCH = 512               # token chunk
NCH = S // TCH          # 4 chunks
DT = D // 128           # 16 D-tiles
KT = S // 128           # 16 kt-tiles
SCALE = float(1.0 / np.sqrt(DH))
N_CORES = 8
GROUPS = [[0, 1, 2, 3], [4, 5, 6, 7]]


def _body(ctx: ExitStack, tc, xT, wq, wk, wv, wo, cosT, sinT,
          out_shard, kT_out, v_out):
    nc = tc.nc
    consts = ctx.enter_context(tc.tile_pool(name="consts", bufs=1))
    persist = ctx.enter_context(tc.tile_pool(name="persist", bufs=1))
    big = ctx.enter_context(tc.tile_pool(name="big", bufs=1))
    scr = ctx.enter_context(tc.tile_pool(name="scr", bufs=2))
    stg = ctx.enter_context(tc.tile_pool(name="stg", bufs=2))
    pp = ctx.enter_context(tc.tile_pool(name="pp", bufs=8, space="PSUM"))
    dram = ctx.enter_context(tc.tile_pool(name="dram", bufs=1, space="DRAM"))
    ph1 = ExitStack()
    wkv = ph1.enter_context(tc.tile_pool(name="wkv", bufs=1))
    trig = ph1.enter_context(tc.tile_pool(name="trig", bufs=1))
    xpool = ph1.enter_context(tc.tile_pool(name="xp", bufs=3))
    rpool = ph1.enter_context(tc.tile_pool(name="rp", bufs=2))

    # ---- constants ----
    ones32 = consts.tile([128, 1], F32)
    nc.gpsimd.memset(ones32[:], 1.0)
    ones = consts.tile([128, 1], F32R)
    nc.scalar.copy(ones[:], ones32[:])
    # additive causal mask for the diagonal 128x128 block in S^T layout:
    # keep (0.0) where kt<=qt i.e. y>=x, else -1e9
    maskadd = consts.tile([128, 128], F32)
    nc.gpsimd.memset(maskadd[:], 0.0)
    nc.gpsimd.affine_select(
        out=maskadd[:], in_=maskadd[:], compare_op=ALU.is_ge, fill=-1e9,
        base=0, pattern=[[1, 128]], channel_multiplier=-1)
    ident = consts.tile([128, 128], F32)
    make_identity(nc, ident[:])

    # ---- weights / rope tables ----
    wq_sb = big.tile([128, DT, NG * DH], F32R, tag="big8k")      # lhsT tiles
    nc.sync.dma_start(wq_sb[:], wq.rearrange("(t p) m -> p t m", p=128))
    wk_sb = wkv.tile([128, DT, DH], F32R)
    nc.sync.dma_start(wk_sb[:], wk.rearrange("(t p) m -> p t m", p=128))
    wv_sb = wkv.tile([128, DT, DH], F32R)
    nc.sync.dma_start(wv_sb[:], wv.rearrange("(t p) m -> p t m", p=128))
    wo_sb = persist.tile([128, NG, D], F32R)
    nc.sync.dma_start(wo_sb[:], wo.rearrange("(c p) e -> p c e", p=128))
    cos_sb = trig.tile([64, S], F32)
    nc.sync.dma_start(cos_sb[:], cosT[:])
    sin_sb = trig.tile([64, S], F32)
    nc.sync.dma_start(sin_sb[:], sinT[:])

    # ---- persistent activations ----
    qT = persist.tile([128, NG, S], F32R)     # roped Q^T per head
    kT = persist.tile([128, S], F32R)         # roped K^T
    vN = persist.tile([128, KT, DH], F32R)    # V natural [t%128, kt, d]

    out_bounce = dram.tile([S, D], F32)
    rs_out = dram.tile([S // 4, D], F32)

    def rope(dst_f32r, psrc, csl, dst32=None):
        """apply rotate-half RoPE to psum tile psrc [128, TCH] using
        cos/sin slice csl; write f32r into dst_f32r ([128, TCH] view); if
        dst32 given, also write fp32 copy there (for exact k output)."""
        c1, s1v = cos_sb[:, csl], sin_sb[:, csl]
        lo_t = rpool.tile([64, TCH], F32, tag="ropeA")
        hi_t = rpool.tile([64, TCH], F32, tag="ropeB")
        t1 = rpool.tile([64, TCH], F32, tag="ropeC")
        # lo = q1*cos - q2*sin ; hi = q1*sin + q2*cos
        nc.vector.tensor_mul(lo_t[:], psrc[0:64, :], c1)
        nc.vector.tensor_mul(t1[:], psrc[64:128, :], s1v)
        nc.vector.tensor_sub(lo_t[:], lo_t[:], t1[:])
        nc.vector.tensor_mul(hi_t[:], psrc[0:64, :], s1v)
        nc.vector.tensor_mul(t1[:], psrc[64:128, :], c1)
        nc.vector.tensor_add(hi_t[:], hi_t[:], t1[:])
        nc.scalar.copy(dst_f32r[0:64, :], lo_t[:])
        nc.scalar.copy(dst_f32r[64:128, :], hi_t[:])
        if dst32 is not None:
            nc.vector.tensor_copy(dst32[0:64, :], lo_t[:])
            nc.vector.tensor_copy(dst32[64:128, :], hi_t[:])

    # ================= phase 1: projections + rope =================
    for c in range(NCH):
        sl = slice(c * TCH, (c + 1) * TCH)
        qps = [pp.tile([128, TCH], F32, tag="ps", name=f"qps{h}")
               for h in range(NG)]
        kps = pp.tile([128, TCH], F32, tag="ps")
        vps = pp.tile([128, TCH], F32, tag="ps")
        for t in range(DT):
            xt = xpool.tile([128, TCH], F32R, tag="x")
            nc.sync.dma_start(xt[:], xT[t * 128:(t + 1) * 128, sl])
            st = (t == 0)
            sp = (t == DT - 1)
            for h in range(NG):
                nc.tensor.matmul(qps[h][:], wq_sb[:, t, h * 128:(h + 1) * 128],
                                 xt[:], start=st, stop=sp)
            nc.tensor.matmul(kps[:], wk_sb[:, t, :], xt[:], start=st, stop=sp)
            nc.tensor.matmul(vps[:], wv_sb[:, t, :], xt[:], start=st, stop=sp)
        for h in range(NG):
            rope(qT[:, h, sl], qps[h], sl)
        k32 = stg.tile([128, TCH], F32, tag="k32")
        rope(kT[:, sl], kps, sl, dst32=k32)
        nc.sync.dma_start(kT_out[:, sl], k32[:])
        # V: psum holds V^T chunk [d, t]; transpose 128-blocks to V natural
        vstage = scr.tile([128, TCH], F32, tag="vstage")
        nc.scalar.copy(vstage[:], vps[:])
        v32 = stg.tile([128, 4, DH], F32, tag="v32")
        for j in range(4):
            trp = pp.tile([128, 128], F32, tag="ps")
            nc.tensor.transpose(trp[:], vstage[:, j * 128:(j + 1) * 128], ident[:])
            nc.scalar.copy(v32[:, j, :], trp[:])
        nc.vector.tensor_copy(vN[:, c * 4:(c + 1) * 4, :], v32[:])
        nc.sync.dma_start(
            v_out.rearrange("(k p) d -> p k d", p=128)[:, c * 4:(c + 1) * 4, :],
            v32[:])

    # ============ phase 2: attention (S^T layout) + out-proj ============
    ph1.close()  # frees wk/wv, cos/sin, x-stream, rope scratch address space
    expool = ctx.enter_context(tc.tile_pool(name="exp", bufs=4))
    ctxT = big.tile([128, NG, S], F32R, tag="big8k")  # reuses wq_sb slot
    for c in range(NCH):
        sl = slice(c * TCH, (c + 1) * TCH)
        nkt = 4 * (c + 1)
        for h in range(NG):
            ctx_ps = pp.tile([128, TCH], F32, tag="ps")
            ones_ps = pp.tile([1, TCH], F32, tag="ps")
            pend = []

            def flush_one():
                k0, ex0, col0 = pend.pop(0)
                nc.tensor.matmul(ones_ps[:, col0:TCH], ones[:], ex0[:, col0:TCH],
                                 start=(k0 == 0), stop=(k0 == nkt - 1))
                nc.tensor.matmul(ctx_ps[:, col0:TCH], vN[:, k0, :],
                                 ex0[:, col0:TCH], start=(k0 == 0),
                                 stop=(k0 == nkt - 1))

            for k in range(nkt):
                j = k - 4 * c
                col0 = 128 * j if j >= 0 else 0
                sc = pp.tile([128, TCH], F32, tag="ps")
                nc.tensor.matmul(sc[:, col0:TCH], kT[:, k * 128:(k + 1) * 128],
                                 qT[:, h, c * TCH + col0:(c + 1) * TCH],
                                 start=True, stop=True)
                if j >= 0:
                    nc.vector.tensor_add(sc[:, col0:col0 + 128],
                                         sc[:, col0:col0 + 128], maskadd[:])
                ex = expool.tile([128, TCH], F32R, tag="exp")
                nc.scalar.activation(ex[:, col0:TCH], sc[:, col0:TCH], AF.Exp,
                                     scale=SCALE)
                pend.append((k, ex, col0))
                if len(pend) > 2:
                    flush_one()
            while pend:
                flush_one()
            rec = scr.tile([1, TCH], F32, tag="rec")
            nc.vector.reciprocal(rec[:], ones_ps[:])
            recb = scr.tile([128, TCH], F32, tag="recb")
            nc.gpsimd.partition_broadcast(recb[:], rec[:])
            nc.vector.tensor_mul(ctxT[:, h, sl], ctx_ps[:], recb[:])

        # out-proj for this chunk's 4 t-tiles (all heads of chunk are done)
        for tt in range(4):
            t0 = (c * 4 + tt) * 128
            for e in range(4):
                op = pp.tile([128, TCH], F32, tag="ps")
                for h in range(NG):
                    nc.tensor.matmul(op[:], ctxT[:, h, t0:t0 + 128],
                                     wo_sb[:, h, e * TCH:(e + 1) * TCH],
                                     start=(h == 0), stop=(h == NG - 1))
                ost = stg.tile([128, TCH], F32, tag="ost")
                nc.scalar.copy(ost[:], op[:])
                nc.sync.dma_start(out_bounce[t0:t0 + 128, e * TCH:(e + 1) * TCH],
                                  ost[:])

    # ============ phase 3: reduce-scatter + final outputs ============
    nc.gpsimd.collective_compute(
        "ReduceScatter", ALU.add, replica_groups=GROUPS,
        ins=[out_bounce[:]], outs=[rs_out[:]])
    nc.sync.dma_start(out_shard[:], rs_out[:])


_NC_CACHE = None


def _build():
    global _NC_CACHE
    if _NC_CACHE is not None:
        return _NC_CACHE
    nc = bacc.Bacc("TRN2", target_bir_lowering=False, debug=False,
                   enable_asserts=False, num_devices=N_CORES)
    xT = nc.dram_tensor("xT", [D, S], F32R, kind="ExternalInput").ap()
    wq = nc.dram_tensor("wq", [D, NG * DH], F32R, kind="ExternalInput").ap()
    wk = nc.dram_tensor("wk", [D, DH], F32R, kind="ExternalInput").ap()
    wv = nc.dram_tensor("wv", [D, DH], F32R, kind="ExternalInput").ap()
    wo = nc.dram_tensor("wo", [NG * DH, D], F32R, kind="ExternalInput").ap()
    cosT = nc.dram_tensor("cosT", [DH // 2, S], F32, kind="ExternalInput").ap()
    sinT = nc.dram_tensor("sinT", [DH // 2, S], F32, kind="ExternalInput").ap()
    out_shard = nc.dram_tensor("out_shard", [S // 4, D], F32,
                               kind="ExternalOutput").ap()
    kT_out = nc.dram_tensor("kT_out", [DH, S], F32, kind="ExternalOutput").ap()
    v_out = nc.dram_tensor("v_out", [S, DH], F32, kind="ExternalOutput").ap()

    with tile.TileContext(nc) as tc:
        with ExitStack() as ctx:
            _body(ctx, tc, xT, wq, wk, wv, wo, cosT, sinT,
                  out_shard, kT_out, v_out)
    nc.compile()
    _NC_CACHE = nc
    return nc


def _in_maps(x, cos, sin, Wq, Wk, Wv, Wo):
    cosT = np.ascontiguousarray(cos.T)
    sinT = np.ascontiguousarray(sin.T)
    maps = []
    for core in range(N_CORES):
        b, g = divmod(core, NKV)
        maps.append({
            "xT": np.ascontiguousarray(x[b].T),
            "wq": np.ascontiguousarray(Wq[:, g * NG * DH:(g + 1) * NG * DH]),
            "wk": np.ascontiguousarray(Wk[:, g * DH:(g + 1) * DH]),
            "wv": np.ascontiguousarray(Wv[:, g * DH:(g + 1) * DH]),
            "wo": np.ascontiguousarray(Wo[g * NG * DH:(g + 1) * NG * DH, :]),
            "cosT": cosT, "sinT": sinT,
        })
    return maps


def run_sharded(x, cos, sin, Wq, Wk, Wv, Wo, trace=False):
    nc = _build()
    maps = _in_maps(x, cos, sin, Wq, Wk, Wv, Wo)
    res = bass_utils.run_bass_kernel_spmd(nc, maps, list(range(N_CORES)),
                                          trace=trace)
    out = np.empty((B, S, D), dtype=np.float32)
    k = np.empty((B, NKV, S, DH), dtype=np.float32)
    v = np.empty((B, NKV, S, DH), dtype=np.float32)
    for core in range(N_CORES):
        b, g = divmod(core, NKV)
        r = res.results[core]
        out[b, g * (S // 4):(g + 1) * (S // 4), :] = r["out_shard"]
        k[b, g] = r["kT_out"].T
        v[b, g] = r["v_out"]
    return (out, k, v), res


def kernel(x, cos, sin, mask, Wq, Wk, Wv, Wo):
    outs, _ = run_sharded(np.asarray(x), np.asarray(cos), np.asarray(sin),
                          np.asarray(Wq), np.asarray(Wk), np.asarray(Wv),
                          np.asarray(Wo))
    return outs


# revision 5
# speedup vs baseline: 1.1077x; 1.1077x over previous
"""GQA attention (B=2,S=2048,D=2048,NH=16,NKV=4,DH=128, RoPE, causal) on 8 trn2 cores.

Sharding: core c = b*4+g handles batch b, kv-head g (4 q-heads).
Per-core: QKV projections (fp32r matmuls), RoPE, flash-style causal attention in
S^T layout (scores^T = K^T.T @ Q^T so softmax denominators come from a ones-matmul
and context^T needs no transposes), out-proj partial, then ReduceScatter(add) over
each batch's 4 cores; host assembles the full output.
"""
import sys, types
sys.path.insert(0, "/opt/trn_rl_repo")

import numpy as np
from contextlib import ExitStack


def _install_ntff_hook():
    try:
        import antenv.axon_hooks  # noqa
        return
    except ImportError:
        pass
    mod = types.ModuleType("antenv.axon_hooks")
    _h = [None]
    mod.set_axon_ntff_profile_hook = lambda h: _h.__setitem__(0, h)
    mod.get_axon_ntff_profile_hook = lambda: _h[0]
    sys.modules["antenv.axon_hooks"] = mod
    try:
        from trn_agent_boot.trn_boot import _ntff_profile_via_ctypes
        mod.set_axon_ntff_profile_hook(
            _ntff_profile_via_ctypes("/opt/axon/libaxon_pjrt.so"))
    except Exception:
        pass


_install_ntff_hook()

import concourse.bass as bass  # noqa: E402
import concourse.mybir as mybir  # noqa: E402
import concourse.tile as tile  # noqa: E402
from concourse import bacc, bass_utils  # noqa: E402
from concourse.masks import make_identity  # noqa: E402

bass_utils.upload_artifacts = lambda tmpdir: tmpdir  # no artifact share here

F32 = mybir.dt.float32
F32R = mybir.dt.float32r
AF = mybir.ActivationFunctionType
ALU = mybir.AluOpType

B, S, D = 2, 2048, 2048
NH, NKV, DH = 16, 4, 128
NG = NH // NKV          # 4 q heads per core
TCH = 512               # token chunk
NCH = S // TCH          # 4 chunks
DT = D // 128           # 16 D-tiles
KT = S // 128           # 16 kt-tiles
SCALE = float(1.0 / np.sqrt(DH))
N_CORES = 8
GROUPS = [[0, 1, 2, 3], [4, 5, 6, 7]]


def _body(ctx: ExitStack, tc, xT, wq, wk, wv, wo, cosT, sinT,
          out_shard, kT_out, v_out):
    nc = tc.nc
    consts = ctx.enter_context(tc.tile_pool(name="consts", bufs=1))
    persist = ctx.enter_context(tc.tile_pool(name="persist", bufs=1))
    big = ctx.enter_context(tc.tile_pool(name="big", bufs=1))
    scr = ctx.enter_context(tc.tile_pool(name="scr", bufs=2))
    stg = ctx.enter_context(tc.tile_pool(name="stg", bufs=2))
    pp = ctx.enter_context(tc.tile_pool(name="pp", bufs=8, space="PSUM"))
    dram = ctx.enter_context(tc.tile_pool(name="dram", bufs=1, space="DRAM"))
    ph1 = ExitStack()
    wkv = ph1.enter_context(tc.tile_pool(name="wkv", bufs=1))
    trig = ph1.enter_context(tc.tile_pool(name="trig", bufs=1))
    xpool = ph1.enter_context(tc.tile_pool(name="xp", bufs=3))
    rpool = ph1.enter_context(tc.tile_pool(name="rp", bufs=2))

    # ---- constants ----
    ones32 = consts.tile([128, 1], F32)
    nc.gpsimd.memset(ones32[:], 1.0)
    ones = consts.tile([128, 1], F32R)
    nc.scalar.copy(ones[:], ones32[:])
    # additive causal mask for the diagonal 128x128 block in S^T layout:
    # keep (0.0) where kt<=qt i.e. y>=x, else -1e9
    maskadd = consts.tile([128, 128], F32)
    nc.gpsimd.memset(maskadd[:], 0.0)
    nc.gpsimd.affine_select(
        out=maskadd[:], in_=maskadd[:], compare_op=ALU.is_ge, fill=-1e9,
        base=0, pattern=[[1, 128]], channel_multiplier=-1)
    ident = consts.tile([128, 128], F32)
    make_identity(nc, ident[:])

    # ---- weights / rope tables ----
    wq_sb = big.tile([128, DT, NG * DH], F32R, tag="big8k")      # lhsT tiles
    nc.sync.dma_start(wq_sb[:], wq.rearrange("(t p) m -> p t m", p=128))
    wk_sb = wkv.tile([128, DT, DH], F32R)
    nc.sync.dma_start(wk_sb[:], wk.rearrange("(t p) m -> p t m", p=128))
    wv_sb = wkv.tile([128, DT, DH], F32R)
    nc.sync.dma_start(wv_sb[:], wv.rearrange("(t p) m -> p t m", p=128))
    wo_sb = persist.tile([128, NG, D], F32R)
    nc.sync.dma_start(wo_sb[:], wo.rearrange("(c p) e -> p c e", p=128))
    cos_sb = trig.tile([64, S], F32)
    nc.sync.dma_start(cos_sb[:], cosT[:])
    sin_sb = trig.tile([64, S], F32)
    nc.sync.dma_start(sin_sb[:], sinT[:])

    # ---- persistent activations ----
    qT = persist.tile([128, NG, S], F32R)     # roped Q^T per head
    kT = persist.tile([128, S], F32R)         # roped K^T
    vN = persist.tile([128, KT, DH], F32R)    # V natural [t%128, kt, d]

    out_bounce = dram.tile([S, D], F32)
    rs_outs = [dram.tile([TCH // 4, D], F32, name=f"rs{c}") for c in range(NCH)]

    def rope(dst_f32r, psrc, csl, dst32=None):
        """apply rotate-half RoPE to psum tile psrc [128, TCH] using
        cos/sin slice csl; write f32r into dst_f32r ([128, TCH] view); if
        dst32 given, also write fp32 copy there (for exact k output)."""
        c1, s1v = cos_sb[:, csl], sin_sb[:, csl]
        lo_t = rpool.tile([64, TCH], F32, tag="ropeA")
        hi_t = rpool.tile([64, TCH], F32, tag="ropeB")
        t1 = rpool.tile([64, TCH], F32, tag="ropeC")
        # lo = q1*cos - q2*sin ; hi = q1*sin + q2*cos
        nc.vector.tensor_mul(lo_t[:], psrc[0:64, :], c1)
        nc.vector.tensor_mul(t1[:], psrc[64:128, :], s1v)
        nc.vector.tensor_sub(lo_t[:], lo_t[:], t1[:])
        nc.vector.tensor_mul(hi_t[:], psrc[0:64, :], s1v)
        nc.vector.tensor_mul(t1[:], psrc[64:128, :], c1)
        nc.vector.tensor_add(hi_t[:], hi_t[:], t1[:])
        nc.scalar.copy(dst_f32r[0:64, :], lo_t[:])
        nc.scalar.copy(dst_f32r[64:128, :], hi_t[:])
        if dst32 is not None:
            nc.vector.tensor_copy(dst32[0:64, :], lo_t[:])
            nc.vector.tensor_copy(dst32[64:128, :], hi_t[:])

    # ================= phase 1: projections + rope =================
    for c in range(NCH):
        sl = slice(c * TCH, (c + 1) * TCH)
        qps = [pp.tile([128, TCH], F32, tag="ps", name=f"qps{h}")
               for h in range(NG)]
        kps = pp.tile([128, TCH], F32, tag="ps")
        vps = pp.tile([128, TCH], F32, tag="ps")
        for t in range(DT):
            xt = xpool.tile([128, TCH], F32R, tag="x")
            nc.sync.dma_start(xt[:], xT[t * 128:(t + 1) * 128, sl])
            st = (t == 0)
            sp = (t == DT - 1)
            for h in range(NG):
                nc.tensor.matmul(qps[h][:], wq_sb[:, t, h * 128:(h + 1) * 128],
                                 xt[:], start=st, stop=sp)
            nc.tensor.matmul(kps[:], wk_sb[:, t, :], xt[:], start=st, stop=sp)
            nc.tensor.matmul(vps[:], wv_sb[:, t, :], xt[:], start=st, stop=sp)
        for h in range(NG):
            rope(qT[:, h, sl], qps[h], sl)
        k32 = stg.tile([128, TCH], F32, tag="k32")
        rope(kT[:, sl], kps, sl, dst32=k32)
        nc.sync.dma_start(kT_out[:, sl], k32[:])
        # V: psum holds V^T chunk [d, t]; transpose 128-blocks to V natural
        vstage = scr.tile([128, TCH], F32, tag="vstage")
        nc.scalar.copy(vstage[:], vps[:])
        v32 = stg.tile([128, 4, DH], F32, tag="v32")
        for j in range(4):
            trp = pp.tile([128, 128], F32, tag="ps")
            nc.tensor.transpose(trp[:], vstage[:, j * 128:(j + 1) * 128], ident[:])
            nc.scalar.copy(v32[:, j, :], trp[:])
        nc.vector.tensor_copy(vN[:, c * 4:(c + 1) * 4, :], v32[:])
        nc.sync.dma_start(
            v_out.rearrange("(k p) d -> p k d", p=128)[:, c * 4:(c + 1) * 4, :],
            v32[:])

    # ============ phase 2: attention (S^T layout) + out-proj ============
    ph1.close()  # frees wk/wv, cos/sin, x-stream, rope scratch address space
    expool = ctx.enter_context(tc.tile_pool(name="exp", bufs=6))
    ctxT = big.tile([128, NG, S], F32R, tag="big8k")  # reuses wq_sb slot
    for c in range(NCH):
        sl = slice(c * TCH, (c + 1) * TCH)
        nkt = 4 * (c + 1)
        for h in range(NG):
            ctx_ps = pp.tile([128, TCH], F32, tag="ps")
            ones_ps = pp.tile([1, TCH], F32, tag="ps")
            pend = []

            def flush_one():
                k0, ex0, col0 = pend.pop(0)
                nc.tensor.matmul(ones_ps[:, col0:TCH], ones[:], ex0[:, col0:TCH],
                                 start=(k0 == 0), stop=(k0 == nkt - 1))
                nc.tensor.matmul(ctx_ps[:, col0:TCH], vN[:, k0, :],
                                 ex0[:, col0:TCH], start=(k0 == 0),
                                 stop=(k0 == nkt - 1))

            for k in range(nkt):
                j = k - 4 * c
                col0 = 128 * j if j >= 0 else 0
                sc = pp.tile([128, TCH], F32, tag="ps")
                nc.tensor.matmul(sc[:, col0:TCH], kT[:, k * 128:(k + 1) * 128],
                                 qT[:, h, c * TCH + col0:(c + 1) * TCH],
                                 start=True, stop=True)
                if j >= 0:
                    nc.vector.tensor_add(sc[:, col0:col0 + 128],
                                         sc[:, col0:col0 + 128], maskadd[:])
                ex = expool.tile([128, TCH], F32R, tag="exp")
                nc.scalar.activation(ex[:, col0:TCH], sc[:, col0:TCH], AF.Exp,
                                     scale=SCALE)
                pend.append((k, ex, col0))
                if len(pend) > 3:
                    flush_one()
            while pend:
                flush_one()
            rec = scr.tile([1, TCH], F32, tag="rec")
            nc.vector.reciprocal_approx_fast(rec[:], ones_ps[:])
            recb = scr.tile([128, TCH], F32, tag="recb")
            nc.gpsimd.partition_broadcast(recb[:], rec[:])
            nc.vector.tensor_mul(ctxT[:, h, sl], ctx_ps[:], recb[:])

        # out-proj for this chunk's 4 t-tiles (all heads of chunk are done)
        for tt in range(4):
            t0 = (c * 4 + tt) * 128
            for e in range(4):
                op = pp.tile([128, TCH], F32, tag="ps")
                for h in range(NG):
                    nc.tensor.matmul(op[:], ctxT[:, h, t0:t0 + 128],
                                     wo_sb[:, h, e * TCH:(e + 1) * TCH],
                                     start=(h == 0), stop=(h == NG - 1))
                ost = stg.tile([128, TCH], F32, tag="ost")
                nc.scalar.copy(ost[:], op[:])
                nc.sync.dma_start(out_bounce[t0:t0 + 128, e * TCH:(e + 1) * TCH],
                                  ost[:])
        # reduce-scatter this chunk's rows while the next chunk computes
        nc.gpsimd.collective_compute(
            "ReduceScatter", ALU.add, replica_groups=GROUPS,
            ins=[out_bounce[c * TCH:(c + 1) * TCH, :]], outs=[rs_outs[c][:]])
        nc.sync.dma_start(out_shard[c * 128:(c + 1) * 128, :], rs_outs[c][:])


_NC_CACHE = None


def _build():
    global _NC_CACHE
    if _NC_CACHE is not None:
        return _NC_CACHE
    nc = bacc.Bacc("TRN2", target_bir_lowering=False, debug=False,
                   enable_asserts=False, num_devices=N_CORES)
    xT = nc.dram_tensor("xT", [D, S], F32R, kind="ExternalInput").ap()
    wq = nc.dram_tensor("wq", [D, NG * DH], F32R, kind="ExternalInput").ap()
    wk = nc.dram_tensor("wk", [D, DH], F32R, kind="ExternalInput").ap()
    wv = nc.dram_tensor("wv", [D, DH], F32R, kind="ExternalInput").ap()
    wo = nc.dram_tensor("wo", [NG * DH, D], F32R, kind="ExternalInput").ap()
    cosT = nc.dram_tensor("cosT", [DH // 2, S], F32, kind="ExternalInput").ap()
    sinT = nc.dram_tensor("sinT", [DH // 2, S], F32, kind="ExternalInput").ap()
    out_shard = nc.dram_tensor("out_shard", [S // 4, D], F32,
                               kind="ExternalOutput").ap()
    kT_out = nc.dram_tensor("kT_out", [DH, S], F32, kind="ExternalOutput").ap()
    v_out = nc.dram_tensor("v_out", [S, DH], F32, kind="ExternalOutput").ap()

    with tile.TileContext(nc) as tc:
        with ExitStack() as ctx:
            _body(ctx, tc, xT, wq, wk, wv, wo, cosT, sinT,
                  out_shard, kT_out, v_out)
    nc.compile()
    _NC_CACHE = nc
    return nc


def _in_maps(x, cos, sin, Wq, Wk, Wv, Wo):
    cosT = np.ascontiguousarray(cos.T)
    sinT = np.ascontiguousarray(sin.T)
    maps = []
    for core in range(N_CORES):
        b, g = divmod(core, NKV)
        maps.append({
            "xT": np.ascontiguousarray(x[b].T),
            "wq": np.ascontiguousarray(Wq[:, g * NG * DH:(g + 1) * NG * DH]),
            "wk": np.ascontiguousarray(Wk[:, g * DH:(g + 1) * DH]),
            "wv": np.ascontiguousarray(Wv[:, g * DH:(g + 1) * DH]),
            "wo": np.ascontiguousarray(Wo[g * NG * DH:(g + 1) * NG * DH, :]),
            "cosT": cosT, "sinT": sinT,
        })
    return maps


def run_sharded(x, cos, sin, Wq, Wk, Wv, Wo, trace=False):
    nc = _build()
    maps = _in_maps(x, cos, sin, Wq, Wk, Wv, Wo)
    res = bass_utils.run_bass_kernel_spmd(nc, maps, list(range(N_CORES)),
                                          trace=trace)
    out = np.empty((B, S, D), dtype=np.float32)
    k = np.empty((B, NKV, S, DH), dtype=np.float32)
    v = np.empty((B, NKV, S, DH), dtype=np.float32)
    for core in range(N_CORES):
        b, g = divmod(core, NKV)
        r = res.results[core]
        for c in range(NCH):
            out[b, c * TCH + g * 128:c * TCH + (g + 1) * 128, :] = \
                r["out_shard"][c * 128:(c + 1) * 128]
        k[b, g] = r["kT_out"].T
        v[b, g] = r["v_out"]
    return (out, k, v), res


def kernel(x, cos, sin, mask, Wq, Wk, Wv, Wo):
    outs, _ = run_sharded(np.asarray(x), np.asarray(cos), np.asarray(sin),
                          np.asarray(Wq), np.asarray(Wk), np.asarray(Wv),
                          np.asarray(Wo))
    return outs


# revision 6
# speedup vs baseline: 1.1244x; 1.0150x over previous
"""GQA attention (B=2,S=2048,D=2048,NH=16,NKV=4,DH=128, RoPE, causal) on 8 trn2 cores.

Sharding: core c = b*4+g handles batch b, kv-head g (4 q-heads).
Per-core: QKV projections (fp32r matmuls), RoPE, flash-style causal attention in
S^T layout (scores^T = K^T.T @ Q^T so softmax denominators come from a ones-matmul
and context^T needs no transposes), out-proj partial, then ReduceScatter(add) over
each batch's 4 cores; host assembles the full output.
"""
import sys, types
sys.path.insert(0, "/opt/trn_rl_repo")

import numpy as np
from contextlib import ExitStack


def _install_ntff_hook():
    try:
        import antenv.axon_hooks  # noqa
        return
    except ImportError:
        pass
    mod = types.ModuleType("antenv.axon_hooks")
    _h = [None]
    mod.set_axon_ntff_profile_hook = lambda h: _h.__setitem__(0, h)
    mod.get_axon_ntff_profile_hook = lambda: _h[0]
    sys.modules["antenv.axon_hooks"] = mod
    try:
        from trn_agent_boot.trn_boot import _ntff_profile_via_ctypes
        mod.set_axon_ntff_profile_hook(
            _ntff_profile_via_ctypes("/opt/axon/libaxon_pjrt.so"))
    except Exception:
        pass


_install_ntff_hook()

import concourse.bass as bass  # noqa: E402
import concourse.mybir as mybir  # noqa: E402
import concourse.tile as tile  # noqa: E402
from concourse import bacc, bass_utils  # noqa: E402
from concourse.masks import make_identity  # noqa: E402

bass_utils.upload_artifacts = lambda tmpdir: tmpdir  # no artifact share here

F32 = mybir.dt.float32
F32R = mybir.dt.float32r
AF = mybir.ActivationFunctionType
ALU = mybir.AluOpType

B, S, D = 2, 2048, 2048
NH, NKV, DH = 16, 4, 128
NG = NH // NKV          # 4 q heads per core
TCH = 512               # token chunk
NCH = S // TCH          # 4 chunks
DT = D // 128           # 16 D-tiles
KT = S // 128           # 16 kt-tiles
SCALE = float(1.0 / np.sqrt(DH))
N_CORES = 8
GROUPS = [[0, 1, 2, 3], [4, 5, 6, 7]]


def _body(ctx: ExitStack, tc, xT, wq, wk, wv, wo, cosT, sinT,
          out_shard, kT_out, v_out):
    nc = tc.nc
    consts = ctx.enter_context(tc.tile_pool(name="consts", bufs=1))
    persist = ctx.enter_context(tc.tile_pool(name="persist", bufs=1))
    big = ctx.enter_context(tc.tile_pool(name="big", bufs=1))
    scr = ctx.enter_context(tc.tile_pool(name="scr", bufs=2))
    stg = ctx.enter_context(tc.tile_pool(name="stg", bufs=2))
    pp = ctx.enter_context(tc.tile_pool(name="pp", bufs=8, space="PSUM"))
    dram = ctx.enter_context(tc.tile_pool(name="dram", bufs=1, space="DRAM"))
    ph1 = ExitStack()
    wkv = ph1.enter_context(tc.tile_pool(name="wkv", bufs=1))
    trig = ph1.enter_context(tc.tile_pool(name="trig", bufs=1))
    xpool = ph1.enter_context(tc.tile_pool(name="xp", bufs=3))
    rpool = ph1.enter_context(tc.tile_pool(name="rp", bufs=2))

    # ---- constants ----
    ones32 = consts.tile([128, 1], F32)
    nc.gpsimd.memset(ones32[:], 1.0)
    ones = consts.tile([128, 1], F32R)
    nc.scalar.copy(ones[:], ones32[:])
    # additive causal mask for the diagonal 128x128 block in S^T layout:
    # keep (0.0) where kt<=qt i.e. y>=x, else -1e9
    maskadd = consts.tile([128, 128], F32)
    nc.gpsimd.memset(maskadd[:], 0.0)
    nc.gpsimd.affine_select(
        out=maskadd[:], in_=maskadd[:], compare_op=ALU.is_ge, fill=-1e9,
        base=0, pattern=[[1, 128]], channel_multiplier=-1)
    ident = consts.tile([128, 128], F32)
    make_identity(nc, ident[:])

    # ---- weights / rope tables ----
    wq_sb = big.tile([128, DT, NG * DH], F32R, tag="big8k")      # lhsT tiles
    wq_r = wq.rearrange("(t p) m -> p t m", p=128)
    wk_sb = wkv.tile([128, DT, DH], F32R)
    nc.sync.dma_start(wk_sb[:], wk.rearrange("(t p) m -> p t m", p=128))
    wv_sb = wkv.tile([128, DT, DH], F32R)
    nc.sync.dma_start(wv_sb[:], wv.rearrange("(t p) m -> p t m", p=128))
    wo_sb = persist.tile([128, NG, D], F32R)
    cos_sb = trig.tile([64, S], F32)
    nc.sync.dma_start(cos_sb[:], cosT[:])
    sin_sb = trig.tile([64, S], F32)
    nc.sync.dma_start(sin_sb[:], sinT[:])

    # ---- persistent activations ----
    qT = persist.tile([128, NG, S], F32R)     # roped Q^T per head
    kT = persist.tile([128, S], F32R)         # roped K^T
    vN = persist.tile([128, KT, DH], F32R)    # V natural [t%128, kt, d]

    out_bounce = dram.tile([S, D], F32)
    rs_outs = [dram.tile([TCH // 4, D], F32, name=f"rs{c}") for c in range(NCH)]

    def rope(dst_f32r, psrc, csl, dst32=None):
        """apply rotate-half RoPE to psum tile psrc [128, TCH] using
        cos/sin slice csl; write f32r into dst_f32r ([128, TCH] view); if
        dst32 given, also write fp32 copy there (for exact k output)."""
        c1, s1v = cos_sb[:, csl], sin_sb[:, csl]
        lo_t = rpool.tile([64, TCH], F32, tag="ropeA")
        hi_t = rpool.tile([64, TCH], F32, tag="ropeB")
        t1 = rpool.tile([64, TCH], F32, tag="ropeC")
        # lo = q1*cos - q2*sin ; hi = q1*sin + q2*cos
        nc.vector.tensor_mul(lo_t[:], psrc[0:64, :], c1)
        nc.vector.tensor_mul(t1[:], psrc[64:128, :], s1v)
        nc.vector.tensor_sub(lo_t[:], lo_t[:], t1[:])
        nc.vector.tensor_mul(hi_t[:], psrc[0:64, :], s1v)
        nc.vector.tensor_mul(t1[:], psrc[64:128, :], c1)
        nc.vector.tensor_add(hi_t[:], hi_t[:], t1[:])
        nc.scalar.copy(dst_f32r[0:64, :], lo_t[:])
        nc.scalar.copy(dst_f32r[64:128, :], hi_t[:])
        if dst32 is not None:
            nc.vector.tensor_copy(dst32[0:64, :], lo_t[:])
            nc.vector.tensor_copy(dst32[64:128, :], hi_t[:])

    # ================= phase 1: projections + rope =================
    for c in range(NCH):
        sl = slice(c * TCH, (c + 1) * TCH)
        qps = [pp.tile([128, TCH], F32, tag="ps", name=f"qps{h}")
               for h in range(NG)]
        kps = pp.tile([128, TCH], F32, tag="ps")
        vps = pp.tile([128, TCH], F32, tag="ps")
        for t in range(DT):
            if c == 0:
                nc.sync.dma_start(wq_sb[:, t, :], wq_r[:, t, :])
            xt = xpool.tile([128, TCH], F32R, tag="x")
            nc.sync.dma_start(xt[:], xT[t * 128:(t + 1) * 128, sl])
            st = (t == 0)
            sp = (t == DT - 1)
            for h in range(NG):
                nc.tensor.matmul(qps[h][:], wq_sb[:, t, h * 128:(h + 1) * 128],
                                 xt[:], start=st, stop=sp)
            nc.tensor.matmul(kps[:], wk_sb[:, t, :], xt[:], start=st, stop=sp)
            nc.tensor.matmul(vps[:], wv_sb[:, t, :], xt[:], start=st, stop=sp)
        for h in range(NG):
            rope(qT[:, h, sl], qps[h], sl)
        k32 = stg.tile([128, TCH], F32, tag="k32")
        rope(kT[:, sl], kps, sl, dst32=k32)
        nc.sync.dma_start(kT_out[:, sl], k32[:])
        # V: psum holds V^T chunk [d, t]; transpose 128-blocks to V natural
        vstage = scr.tile([128, TCH], F32, tag="vstage")
        nc.scalar.copy(vstage[:], vps[:])
        v32 = stg.tile([128, 4, DH], F32, tag="v32")
        for j in range(4):
            trp = pp.tile([128, 128], F32, tag="ps")
            nc.tensor.transpose(trp[:], vstage[:, j * 128:(j + 1) * 128], ident[:])
            nc.scalar.copy(v32[:, j, :], trp[:])
        nc.vector.tensor_copy(vN[:, c * 4:(c + 1) * 4, :], v32[:])
        nc.sync.dma_start(
            v_out.rearrange("(k p) d -> p k d", p=128)[:, c * 4:(c + 1) * 4, :],
            v32[:])

    # ============ phase 2: attention (S^T layout) + out-proj ============
    ph1.close()  # frees wk/wv, cos/sin, x-stream, rope scratch address space
    nc.sync.dma_start(wo_sb[:], wo.rearrange("(c p) e -> p c e", p=128))
    expool = ctx.enter_context(tc.tile_pool(name="exp", bufs=6))
    ctxT = big.tile([128, NG, S], F32R, tag="big8k")  # reuses wq_sb slot
    for c in range(NCH):
        sl = slice(c * TCH, (c + 1) * TCH)
        nkt = 4 * (c + 1)
        for h in range(NG):
            ctx_ps = pp.tile([128, TCH], F32, tag="ps")
            ones_ps = pp.tile([1, TCH], F32, tag="ps")
            pend = []

            def flush_one():
                k0, ex0, col0 = pend.pop(0)
                nc.tensor.matmul(ones_ps[:, col0:TCH], ones[:], ex0[:, col0:TCH],
                                 start=(k0 == 0), stop=(k0 == nkt - 1))
                nc.tensor.matmul(ctx_ps[:, col0:TCH], vN[:, k0, :],
                                 ex0[:, col0:TCH], start=(k0 == 0),
                                 stop=(k0 == nkt - 1))

            for k in range(nkt):
                j = k - 4 * c
                col0 = 128 * j if j >= 0 else 0
                sc = pp.tile([128, TCH], F32, tag="ps")
                nc.tensor.matmul(sc[:, col0:TCH], kT[:, k * 128:(k + 1) * 128],
                                 qT[:, h, c * TCH + col0:(c + 1) * TCH],
                                 start=True, stop=True)
                if j >= 0:
                    nc.vector.tensor_add(sc[:, col0:col0 + 128],
                                         sc[:, col0:col0 + 128], maskadd[:])
                ex = expool.tile([128, TCH], F32R, tag="exp")
                nc.scalar.activation(ex[:, col0:TCH], sc[:, col0:TCH], AF.Exp,
                                     scale=SCALE)
                pend.append((k, ex, col0))
                if len(pend) > 3:
                    flush_one()
            while pend:
                flush_one()
            rec = scr.tile([1, TCH], F32, tag="rec")
            nc.vector.reciprocal_approx_fast(rec[:], ones_ps[:])
            recb = scr.tile([128, TCH], F32, tag="recb")
            nc.gpsimd.partition_broadcast(recb[:], rec[:])
            nc.vector.tensor_mul(ctxT[:, h, sl], ctx_ps[:], recb[:])

        # out-proj for this chunk's 4 t-tiles (all heads of chunk are done)
        for tt in range(4):
            t0 = (c * 4 + tt) * 128
            for e in range(4):
                op = pp.tile([128, TCH], F32, tag="ps")
                for h in range(NG):
                    nc.tensor.matmul(op[:], ctxT[:, h, t0:t0 + 128],
                                     wo_sb[:, h, e * TCH:(e + 1) * TCH],
                                     start=(h == 0), stop=(h == NG - 1))
                ost = stg.tile([128, TCH], F32, tag="ost")
                nc.scalar.copy(ost[:], op[:])
                nc.sync.dma_start(out_bounce[t0:t0 + 128, e * TCH:(e + 1) * TCH],
                                  ost[:])
        # reduce-scatter this chunk's rows while the next chunk computes
        nc.gpsimd.collective_compute(
            "ReduceScatter", ALU.add, replica_groups=GROUPS,
            ins=[out_bounce[c * TCH:(c + 1) * TCH, :]], outs=[rs_outs[c][:]])
        nc.sync.dma_start(out_shard[c * 128:(c + 1) * 128, :], rs_outs[c][:])


_NC_CACHE = None


def _build():
    global _NC_CACHE
    if _NC_CACHE is not None:
        return _NC_CACHE
    nc = bacc.Bacc("TRN2", target_bir_lowering=False, debug=False,
                   enable_asserts=False, num_devices=N_CORES)
    xT = nc.dram_tensor("xT", [D, S], F32R, kind="ExternalInput").ap()
    wq = nc.dram_tensor("wq", [D, NG * DH], F32R, kind="ExternalInput").ap()
    wk = nc.dram_tensor("wk", [D, DH], F32R, kind="ExternalInput").ap()
    wv = nc.dram_tensor("wv", [D, DH], F32R, kind="ExternalInput").ap()
    wo = nc.dram_tensor("wo", [NG * DH, D], F32R, kind="ExternalInput").ap()
    cosT = nc.dram_tensor("cosT", [DH // 2, S], F32, kind="ExternalInput").ap()
    sinT = nc.dram_tensor("sinT", [DH // 2, S], F32, kind="ExternalInput").ap()
    out_shard = nc.dram_tensor("out_shard", [S // 4, D], F32,
                               kind="ExternalOutput").ap()
    kT_out = nc.dram_tensor("kT_out", [DH, S], F32, kind="ExternalOutput").ap()
    v_out = nc.dram_tensor("v_out", [S, DH], F32, kind="ExternalOutput").ap()

    with tile.TileContext(nc) as tc:
        with ExitStack() as ctx:
            _body(ctx, tc, xT, wq, wk, wv, wo, cosT, sinT,
                  out_shard, kT_out, v_out)
    nc.compile()
    _NC_CACHE = nc
    return nc


def _in_maps(x, cos, sin, Wq, Wk, Wv, Wo):
    cosT = np.ascontiguousarray(cos.T)
    sinT = np.ascontiguousarray(sin.T)
    maps = []
    for core in range(N_CORES):
        b, g = divmod(core, NKV)
        maps.append({
            "xT": np.ascontiguousarray(x[b].T),
            "wq": np.ascontiguousarray(Wq[:, g * NG * DH:(g + 1) * NG * DH]),
            "wk": np.ascontiguousarray(Wk[:, g * DH:(g + 1) * DH]),
            "wv": np.ascontiguousarray(Wv[:, g * DH:(g + 1) * DH]),
            "wo": np.ascontiguousarray(Wo[g * NG * DH:(g + 1) * NG * DH, :]),
            "cosT": cosT, "sinT": sinT,
        })
    return maps


def run_sharded(x, cos, sin, Wq, Wk, Wv, Wo, trace=False):
    nc = _build()
    maps = _in_maps(x, cos, sin, Wq, Wk, Wv, Wo)
    res = bass_utils.run_bass_kernel_spmd(nc, maps, list(range(N_CORES)),
                                          trace=trace)
    out = np.empty((B, S, D), dtype=np.float32)
    k = np.empty((B, NKV, S, DH), dtype=np.float32)
    v = np.empty((B, NKV, S, DH), dtype=np.float32)
    for core in range(N_CORES):
        b, g = divmod(core, NKV)
        r = res.results[core]
        for c in range(NCH):
            out[b, c * TCH + g * 128:c * TCH + (g + 1) * 128, :] = \
                r["out_shard"][c * 128:(c + 1) * 128]
        k[b, g] = r["kT_out"].T
        v[b, g] = r["v_out"]
    return (out, k, v), res


def kernel(x, cos, sin, mask, Wq, Wk, Wv, Wo):
    outs, _ = run_sharded(np.asarray(x), np.asarray(cos), np.asarray(sin),
                          np.asarray(Wq), np.asarray(Wk), np.asarray(Wv),
                          np.asarray(Wo))
    return outs
